# revision 2
# baseline (speedup 1.0000x reference)
"""Trainium2 Bass kernel v2 for nn_BagKQMClassModel.

Math (per batch item b):
    K2[b,n,m] = exp((2*g - a2[b,n] - b2[m]) / s^2),  g = a[b,n]:c[m]
    out_w[b,m] ~ sum_n K2;  probs = (out_w @ Wnum) / (out_w @ Wden)

Design (per core, 32 items x 128 n = 4096 cols item-major, m on partitions):
  * mm1: fp8e4m3 DoubleRow (K=34 as 17x2: 32 dims + a2-row + pad), 4-way
    row-packed via tile_position (0/32/64/96, 0) -- four concurrent streams
    in the PE array, ~4 cols/cycle.  Operands staged host-side already
    transposed/augmented/paired (layout staging).
  * b2 folded into the exp bias (per-partition ACT bias / Schraudolph bias).
  * exp split across two engines reading pool-rotated [128,1024] PSUM tiles
    (bufs=3 -> 6 banks; framework per-buffer WAR fences give pipelining):
      - ACT: exact exp -> fp8e4m3 K2
      - DVE: Schraudolph bit-trick exp: round(A8*x + B8) as int8 IS the
        fp8e4m3 bit pattern of ~exp(x) (one tensor_scalar per element).
  * mm2: fp8 DoubleRow contracts m-chunk PAIRS (K=256) with W [128,2,11],
    PSUM-overlay accumulation over n does the bag-reduction for free:
    all 64 matmuls accumulate into one S [11, 512] (cols = 32 items x 16
    n-residues).
  * PE warmup burst at start trips the HAM clock gate to 2.4 GHz.
  * epilogue: reduce n-residues, transpose, divide -> probs [32, 10].

Sharding: batch 256 -> 32 items/core across 8 cores; c_x/c_y/comp_w
replicated; no collectives.
"""

import math
import numpy as np
from contextlib import ExitStack

import concourse.bacc as bacc
import concourse.mybir as mybir
import concourse.tile as tile
from concourse.bass import ts
from concourse.bass_utils import run_bass_kernel_spmd

NCORES = 8
BS, N, DX, DY, M = 256, 128, 32, 10, 2048
BPC = BS // NCORES          # 32 items per core
BN = BPC * N                # 4096 free columns per core
MB = M // 128               # 16 m-chunks
KAUG = DX + 2               # 34 contraction rows (a^T, -a2/2, 0-pad)
NBLK = BN // 512            # 8 col-blocks of 512 per chunk
RSLOTS = 7                  # psum ring slots (7 banks); 1 bank for S
RUNCAP = 4                  # max ring slots per exp instruction
NRES = 16                   # n-residues after overlay (128 n -> 16)
MIN_SIGMA = 1e-3

FP32 = mybir.dt.float32
BF16 = mybir.dt.bfloat16
FP8 = mybir.dt.float8e4
I8 = mybir.dt.int8
AX = mybir.AxisListType
ALU = mybir.AluOpType
ACTF = mybir.ActivationFunctionType
DR = mybir.MatmulPerfMode.DoubleRow

NP_BF16 = np.dtype(mybir.dt.np(BF16))
NP_FP8 = np.dtype(mybir.dt.np(FP8))

# Schraudolph fp8e4m3: bitpattern = round(A8*ln(K2) + B8); C_ADJ tuned so the
# mean multiplicative error ~0 given round-to-nearest fp32->int8 conversion.
A8 = 8.0 / math.log(2.0)
C_ADJ = -0.5


def _exp_schedule():
    """Static engine assignment per (mb, h) 1024-col tile: greedy balance
    with measured rates (ACT (N+312)/1.2, DVE (N+79)/0.96, DVE ~2.5us of
    prologue chains)."""
    c_act = (1024 + 312) / 1.2
    c_dve = (1024 + 79) / 0.96
    t_act, t_dve = 0.0, 2500.0
    sched = []
    for _ in range(MB * 4):
        if t_act + c_act <= t_dve + c_dve:
            sched.append("act")
            t_act += c_act
        else:
            sched.append("dve")
            t_dve += c_dve
    return sched


def _body(tc, atd, ctd, consts, out_d, scale, s1_schrau, b8_eff):
    nc = tc.nc
    with ExitStack() as ctx:
        const = ctx.enter_context(tc.tile_pool(name="const", bufs=1))
        work = ctx.enter_context(tc.tile_pool(name="work", bufs=2))

        K2 = const.tile([128, MB, BN], FP8)

        # ---- operand loads (host staged, transposed+augmented, fp8 DR
        # layout [17, 2, .]); 4 copies at partition bases 0/32/64/96 for
        # 4-way row-tiling ----------------------------------------------
        ATD = const.tile([128, 2, BN], FP8)
        CTD = const.tile([128, 2, M], FP8)
        blob = const.tile([128, 1792], mybir.dt.uint8)
        # ALL loads on the sync queue, full 128-partition-wide transfers
        # (host bakes the 4 row-tiling copies at partition bases 0/32/64/96,
        # and packs cx/cy/cw/identity into one consts blob).  The scalar
        # queue is the ACT engine -- keep it free for exps; gpsimd DMAs pay
        # ~1.7us drains.  AT streams in col-chunks so mm1 starts early.
        nc.sync.dma_start(out=CTD, in_=ctd)
        nc.sync.dma_start(out=blob, in_=consts)
        for c in range(4):
            nc.sync.dma_start(
                out=ATD[:, :, ts(c, BN // 4)], in_=atd[:, :, ts(c, BN // 4)]
            )
        cx_sb = blob[:, 0:1024].bitcast(BF16).rearrange(
            "p (t d) -> p t d", d=DX
        )
        cy_sb = blob[:, 1024:1664].bitcast(FP32).rearrange(
            "p (t d) -> p t d", d=DY
        )
        cw_sb = blob[:, 1664:1728].bitcast(FP32)
        identity = blob[0 : DY + 1, 1728:1772].bitcast(FP32)

        # ---- b2 and exp biases --------------------------------------------
        sqx = work.tile([128, MB, DX], FP32, tag="sqx")
        nc.vector.tensor_mul(sqx, cx_sb, cx_sb)
        b2 = const.tile([128, MB], FP32)
        nc.vector.tensor_reduce(out=b2, in_=sqx, axis=AX.X, op=ALU.add)
        biasE = const.tile([128, MB], FP32)
        nc.vector.tensor_scalar_mul(biasE, b2, -0.5 * scale)
        biasS = const.tile([128, MB], FP32)
        nc.vector.tensor_scalar(
            out=biasS, in0=b2, scalar1=-0.5 * scale * A8, scalar2=b8_eff,
            op0=ALU.mult, op1=ALU.add,
        )

        # ---- W build (deferred into the loop so DVE's first exps aren't
        # queued behind it): [128, 16, 16] fp8, cols 0:10 = cw*M*cyhat^2,
        # col 10 = cw*M, cols 11:16 zero-pad (DoubleRow needs 16B ko-step) ---
        WP = const.tile([128, MB, 16], FP8)

        def build_W():
            sqy = work.tile([128, MB, DY], FP32, tag="sqy")
            nc.vector.tensor_mul(sqy, cy_sb, cy_sb)
            ssum = work.tile([128, MB], FP32, tag="ssum")
            nc.vector.tensor_reduce(out=ssum, in_=sqy, axis=AX.X, op=ALU.add)
            rec = work.tile([128, MB], FP32, tag="rec")
            nc.vector.reciprocal(rec, ssum)
            cwm = work.tile([128, MB], FP32, tag="cwm")
            nc.vector.tensor_scalar_mul(cwm, cw_sb, float(M))
            fac = work.tile([128, MB], FP32, tag="fac")
            nc.vector.tensor_mul(fac, rec, cwm)
            WF = work.tile([128, MB, 16], FP32, tag="wf")
            nc.vector.memset(WF, 0.0)
            fac_b = fac.rearrange("p (t one) -> p t one", one=1).broadcast_to(
                [128, MB, DY]
            )
            nc.vector.tensor_mul(WF[:, :, 0:DY], sqy, fac_b)
            nc.vector.tensor_copy(
                WF[:, :, DY : DY + 1],
                cwm.rearrange("p (t one) -> p t one", one=1),
            )
            nc.vector.tensor_copy(WP, WF)

        # ---- main loop -----------------------------------------------------
        sched = _exp_schedule()

        with ExitStack() as lctx:
            actp = lctx.enter_context(
                tc.tile_pool(name="actps", space="PSUM", bufs=2)
            )
            dvep = lctx.enter_context(
                tc.tile_pool(name="dveps", space="PSUM", bufs=2)
            )
            # PE warmup: back-to-back dummy matmuls gated on the CTD load
            # trip the HAM clock gate to 2.4 GHz right before the real work.
            warm = actp.tile([128, 1024], FP32, tag="ga", name="warm")
            for _ in range(8):
                nc.tensor.matmul(
                    warm[:, 0:512], CTD[0:17, :, 0:128], CTD[0:17, :, 0:512],
                    start=True, stop=True, perf_mode=DR,
                )

            # ---- phase 1: mm1 + exp -> K2 (PE does only mm1; short, evenly
            # spread PE gaps keep the HAM clock warm) ------------------------
            for mb in range(MB):
                for h in range(4):
                    eng = sched[mb * 4 + h]
                    pool, tag = (actp, "ga") if eng == "act" else (dvep, "gd")
                    g = pool.tile([128, 1024], FP32, tag=tag)
                    for qq in range(2):
                        q = 2 * h + qq
                        b = 32 * (q % 4)
                        nc.tensor.matmul(
                            g[:, ts(qq, 512)],
                            CTD[b : b + 17, :, ts(mb, 128)],
                            ATD[b : b + 17, :, ts(q, 512)],
                            start=True, stop=True, perf_mode=DR,
                            tile_position=(b, 0),
                        )
                    dst = K2[:, mb, h * 1024 : (h + 1) * 1024]
                    if eng == "act":
                        nc.scalar.activation(
                            dst, g, ACTF.Exp,
                            bias=biasE[:, mb : mb + 1], scale=scale,
                        )
                    else:
                        nc.vector.tensor_scalar(
                            out=dst.bitcast(I8), in0=g,
                            scalar1=s1_schrau, scalar2=biasS[:, mb : mb + 1],
                            op0=ALU.mult, op1=ALU.add,
                        )
                if mb == 0:
                    build_W()

        # ---- phase 2: all mm2s as one dense warm PE burst ------------------
        with ExitStack() as lctx:
            spool = lctx.enter_context(
                tc.tile_pool(name="sps", space="PSUM", bufs=1)
            )
            S = spool.tile([DY + 1, 512], FP32, name="S")
            K2v = K2.rearrange("p t (i n) -> p t i n", i=BPC)
            for p in range(MB // 2):
                for gg in range(8):
                    nc.tensor.matmul(
                        S,
                        WP[:, 2 * p : 2 * p + 2, 0 : DY + 1],
                        K2v[:, 2 * p : 2 * p + 2, :, ts(gg, NRES)],
                        start=(p == 0 and gg == 0),
                        stop=(p == MB // 2 - 1 and gg == 7),
                        perf_mode=DR,
                    )

            # ---- epilogue: T = reduce_n S; probs = T[:10]/T[10] ------------
            Tsb = const.tile([DY + 1, BPC], FP32)
            nc.vector.tensor_reduce(
                out=Tsb,
                in_=S.rearrange("p (i r) -> p i r", r=NRES),
                axis=AX.X, op=ALU.add,
            )

        with tc.tile_pool(name="epips", space="PSUM", bufs=1) as epi:
            trT = epi.tile([BPC, DY + 1], FP32, name="trT")
            nc.tensor.transpose(trT, Tsb, identity)
            Tt = const.tile([BPC, DY + 1], FP32)
            nc.vector.tensor_copy(Tt, trT)
            recd = const.tile([BPC, 1], FP32)
            nc.vector.reciprocal(recd, Tt[:, DY : DY + 1])
            outsb = const.tile([BPC, DY], FP32)
            nc.vector.tensor_scalar(
                out=outsb, in0=Tt[:, 0:DY], scalar1=recd, scalar2=None,
                op0=ALU.mult,
            )
            nc.sync.dma_start(out=out_d, in_=outsb)


def build_program(scale, s1_schrau, b8_eff):
    nc = bacc.Bacc(
        "TRN2", target_bir_lowering=False, debug=False,
        enable_asserts=False, num_devices=NCORES,
    )
    atd = nc.dram_tensor("at_aug", [128, 2, BN], FP8, kind="ExternalInput").ap()
    ctd = nc.dram_tensor("ct_aug", [128, 2, M], FP8, kind="ExternalInput").ap()
    consts = nc.dram_tensor("consts", [128, 1792], mybir.dt.uint8,
                            kind="ExternalInput").ap()
    out = nc.dram_tensor("out", [BPC, DY], FP32, kind="ExternalOutput").ap()
    with tile.TileContext(nc) as tc:
        _body(tc, atd, ctd, consts, out, scale, s1_schrau, b8_eff)
    nc.compile()
    return nc


_PROGRAM_CACHE: dict = {}


def _get_program(sigma_f):
    key = float(sigma_f)
    nc = _PROGRAM_CACHE.get(key)
    if nc is None:
        s = max(key, MIN_SIGMA)
        scale = 2.0 / (s * s)
        s1 = A8 * scale                     # applied to h (=g - a2/2)
        b8 = 56.0 + C_ADJ
        nc = build_program(scale, s1, b8)
        _PROGRAM_CACHE[key] = nc
    return nc


def make_in_maps(inputs, c_x, c_y, comp_w):
    """Host staging: shard batch, transpose+augment mm1 operands."""
    shards = inputs.reshape(NCORES, BPC, N, DX)
    cxc = np.ascontiguousarray(c_x)
    ct = np.empty((KAUG, M), dtype=np.float32)
    ct[0:DX] = cxc.T
    ct[DX] = 1.0
    ct[DX + 1] = 0.0
    # DoubleRow pairing: contraction row k = ko*17 + ki
    ct17 = ct.reshape(2, 17, M).transpose(1, 0, 2).astype(NP_FP8)
    ctc = np.zeros((128, 2, M), dtype=NP_FP8)
    for s in range(4):
        ctc[32 * s : 32 * s + 17] = ct17
    ctc = np.ascontiguousarray(ctc)
    blob = np.zeros((128, 1792), dtype=np.uint8)
    blob[:, 0:1024] = (
        c_x.reshape(MB, 128, DX).transpose(1, 0, 2).astype(NP_BF16)
        .copy().reshape(128, MB * DX).view(np.uint8)
    )
    blob[:, 1024:1664] = (
        c_y.reshape(MB, 128, DY).transpose(1, 0, 2).astype(np.float32)
        .copy().reshape(128, MB * DY).view(np.uint8)
    )
    blob[:, 1664:1728] = (
        comp_w.reshape(MB, 128).T.astype(np.float32).copy().reshape(128, MB)
        .view(np.uint8)
    )
    blob[0 : DY + 1, 1728:1772] = (
        np.eye(DY + 1, dtype=np.float32).view(np.uint8)
    )
    blob = np.ascontiguousarray(blob)

    in_maps = []
    for i in range(NCORES):
        a = shards[i].reshape(BPC * N, DX).astype(np.float32)
        at = np.empty((KAUG, BN), dtype=np.float32)
        at[0:DX] = a.T
        at[DX] = -0.5 * (a * a).sum(axis=1)
        at[DX + 1] = 0.0
        at17 = at.reshape(2, 17, BN).transpose(1, 0, 2).astype(NP_FP8)
        at8 = np.zeros((128, 2, BN), dtype=NP_FP8)
        for s in range(4):
            at8[32 * s : 32 * s + 17] = at17
        at8 = np.ascontiguousarray(at8)
        in_maps.append(
            {
                "at_aug": at8,
                "ct_aug": ctc,
                "consts": blob,
            }
        )
    return in_maps


def kernel(inputs, sigma, c_x, c_y, comp_w, _run_kwargs=None):
    sigma_f = float(np.asarray(sigma, dtype=np.float64))
    nc = _get_program(sigma_f)
    in_maps = make_in_maps(
        np.asarray(inputs), np.asarray(c_x), np.asarray(c_y), np.asarray(comp_w)
    )
    res = run_bass_kernel_spmd(
        nc, in_maps, core_ids=list(range(NCORES)), **(_run_kwargs or {})
    )
    out = np.concatenate([res.results[i]["out"] for i in range(NCORES)], axis=0)
    return out.astype(np.float32)
